# revision 4
# baseline (speedup 1.0000x reference)
"""Trainium2 Bass kernel for CharOffsetAttention (RoPE attention block).

Sharding (8 cores): data-parallel over batch (B=4 -> pairs of cores) x
tensor-parallel over heads (16 heads -> 8 per core).  Each core computes
qkv projections for its 8 heads, rope, causal attention, and a partial
output projection; the host sums the two head-half partials per batch.

On-device layout is head-dim-major throughout ("S-transposed" scheme):
  qT/kT: [512 head-dims, 2048 tokens]  (rope pairs deinterleaved host-side)
  scores: S.T[k_token, q_token] per head, causal blocks only
  out:    attT is directly the lhsT of the wo matmul

This version exploits PE array tiling concurrency (measured 2x on HW):
  - scores: the two heads of a pair run CONCURRENTLY as row-tiles
    (tile_position (0,0)/(64,0), K=64 each) writing the two halves of one
    [128,1024] PSUM tile, so a single wide exp covers both heads.
  - PV: the two heads run concurrently as col-tiles ((0,0)/(0,64), M=64)
    into one PSUM bank whose partition layout IS the attT layout.
  - softmax denominators: ones-lhsT M=1 matmuls, col-tiled into a shared
    bank (partitions 0/32); reciprocal via reciprocal_approx_fast.
  - qkv / wo projection matmul chains are woven into the attention loop's
    spare slots so the PE never idles and HAM stays un-throttled.
All matmuls are bf16 with f32 PSUM accumulation.
"""
import sys
from collections import deque

if '/opt/trn_rl_repo' not in sys.path:
    sys.path.insert(0, '/opt/trn_rl_repo')

import numpy as np
import ml_dtypes

import concourse.bass as bass
import concourse.bacc as bacc
import concourse.tile as tile
import concourse.mybir as mybir
from concourse.bass_utils import run_bass_kernel_spmd

F32 = mybir.dt.float32
BF16 = mybir.dt.bfloat16
NPBF16 = ml_dtypes.bfloat16

# full-problem constants
B, T, D_MODEL, N_HEADS, HEAD_DIM = 4, 2048, 1024, 16, 64
N_CORES = 8


def build_program(t=T, din=D_MODEL, nhc=N_HEADS // 2, debug=False):
    """Build the per-core SPMD program."""
    hd = HEAD_DIM                    # 64
    dh = nhc * hd                    # 512 head dims on this core
    nkc = t // 128                   # 16 k-token chunks
    nnq = t // 512                   # 4 q-blocks
    npair = nhc // 2                 # 4 head pairs
    ndin = din // 128                # 8 contraction tiles
    scale = 1.0 / np.sqrt(hd)

    nc = bacc.Bacc("TRN2", target_bir_lowering=False, debug=debug,
                   num_devices=N_CORES)

    xT_d = nc.dram_tensor("xT", [din, t], BF16, kind="ExternalInput")
    wqT_d = nc.dram_tensor("wqT", [din, dh], BF16, kind="ExternalInput")
    wkT_d = nc.dram_tensor("wkT", [din, dh], BF16, kind="ExternalInput")
    wvT_d = nc.dram_tensor("wvT", [din, dh], BF16, kind="ExternalInput")
    woT_d = nc.dram_tensor("woT", [dh, din], BF16, kind="ExternalInput")
    cos_d = nc.dram_tensor("cosr", [128, t], BF16, kind="ExternalInput")
    sinp_d = nc.dram_tensor("sinp", [128, t], BF16, kind="ExternalInput")
    sinn_d = nc.dram_tensor("sinn", [128, t], BF16, kind="ExternalInput")
    out_d = nc.dram_tensor("out", [t, din], F32, kind="ExternalOutput")

    GE = mybir.AluOpType.is_ge
    EXP = mybir.ActivationFunctionType.Exp

    with tile.TileContext(nc) as tc:
        with tc.tile_pool(name="persist", bufs=1) as pp:
            cos_sb = pp.tile([128, t], BF16, tag="cos")
            sinp_sb = pp.tile([128, t], BF16, tag="sinp")
            sinn_sb = pp.tile([128, t], BF16, tag="sinn")
            nc.sync.dma_start(cos_sb[:], cos_d[:])
            nc.sync.dma_start(sinp_sb[:], sinp_d[:])
            nc.sync.dma_start(sinn_sb[:], sinn_d[:])

            woT_sb = [pp.tile([128, din], BF16, tag=f"woT{i}", name=f"woT{i}")
                      for i in range(npair)]
            for i in range(npair):
                nc.sync.dma_start(woT_sb[i][:], woT_d[i * 128:(i + 1) * 128, :])

            qT = [pp.tile([128, t], BF16, tag=f"qT{c}", name=f"qT{c}")
                  for c in range(npair)]
            kT = [pp.tile([128, t], BF16, tag=f"kT{c}", name=f"kT{c}")
                  for c in range(npair)]
            attT = [pp.tile([128, t], BF16, tag=f"attT{c}", name=f"attT{c}")
                    for c in range(npair)]
            v_sb = [pp.tile([128, dh], BF16, tag=f"v{i}", name=f"v{i}")
                    for i in range(nkc)]
            # ones for the denominator (K=128,M=1) and broadcast (K=1,M=64)
            onesd = pp.tile([128, 1], BF16, tag="onesd")
            onesbc = pp.tile([128, 64], BF16, tag="onesbc")
            nc.gpsimd.memset(onesd[:], 1.0)
            nc.gpsimd.memset(onesbc[:], 1.0)
            rcf = pp.tile([128, 512], F32, tag="rcf")
            rc2 = pp.tile([128, 512], BF16, tag="rc2")
            bcs = pp.tile([128, 512], F32, tag="bcs")

            with (
                tc.tile_pool(name="pin", bufs=1) as pin,
                tc.tile_pool(name="praw", bufs=4) as praw,
                tc.tile_pool(name="rtp", bufs=2) as rtp,
                tc.tile_pool(name="exps", bufs=5) as exps,
                tc.tile_pool(name="osb", bufs=3) as osb,
                tc.tile_pool(name="stp", bufs=2, space="PSUM") as stp,
                tc.tile_pool(name="atp", bufs=2, space="PSUM") as atp,
                tc.tile_pool(name="dnb", bufs=1, space="PSUM") as dnb,
                tc.tile_pool(name="msc", bufs=1, space="PSUM") as msc,
            ):
                xT_sb = [pin.tile([128, t], BF16, tag=f"xT{k}", name=f"xTs{k}")
                         for k in range(ndin)]
                wqT_sb = [pin.tile([128, dh], BF16, tag=f"wqT{k}",
                                   name=f"wqTs{k}") for k in range(ndin)]
                wkT_sb = [pin.tile([128, dh], BF16, tag=f"wkT{k}",
                                   name=f"wkTs{k}") for k in range(ndin)]
                wvT_sb = [pin.tile([128, dh], BF16, tag=f"wvT{k}",
                                   name=f"wvTs{k}") for k in range(ndin)]
                for k in range(ndin):
                    sl = slice(k * 128, (k + 1) * 128)
                    nc.sync.dma_start(xT_sb[k][:], xT_d[sl, :])
                    nc.sync.dma_start(wqT_sb[k][:], wqT_d[sl, :])
                    nc.sync.dma_start(wkT_sb[k][:], wkT_d[sl, :])
                    nc.sync.dma_start(wvT_sb[k][:], wvT_d[sl, :])

                # ---------- projection chain emitters ----------
                def qk_chain(w_sb, raw, c, n):
                    ps = msc.tile([128, 512], F32, tag="msc")
                    for kk in range(ndin):
                        nc.tensor.matmul(
                            ps[:], w_sb[kk][:, c * 128:(c + 1) * 128],
                            xT_sb[kk][:, n * 512:(n + 1) * 512],
                            start=(kk == 0), stop=(kk == ndin - 1))
                    nc.vector.tensor_copy(raw[:, n * 512:(n + 1) * 512], ps[:])

                def v_chain(tt):
                    ps = msc.tile([128, 512], F32, tag="msc")
                    for kk in range(ndin):
                        nc.tensor.matmul(
                            ps[:], xT_sb[kk][:, tt * 128:(tt + 1) * 128],
                            wvT_sb[kk][:],
                            start=(kk == 0), stop=(kk == ndin - 1))
                    nc.scalar.copy(v_sb[tt][:], ps[:])

                def wo_chain(tt, n2):
                    ps = msc.tile([128, 512], F32, tag="msc")
                    for kk in range(npair):
                        nc.tensor.matmul(
                            ps[:], attT[kk][:, tt * 128:(tt + 1) * 128],
                            woT_sb[kk][:, n2:n2 + 512],
                            start=(kk == 0), stop=(kk == npair - 1))
                    o = osb.tile([128, 512], F32, tag="osb")
                    nc.vector.tensor_copy(o[:], ps[:])
                    nc.sync.dma_start(out_d[tt * 128:(tt + 1) * 128,
                                            n2:n2 + 512], o[:])

                def rope(raw, dst):
                    # re' = re*cos - im*sin ; im' = im*cos + re*sin
                    # (sin tables repeat every 32 partitions; slice at the
                    #  source operand's base partition)
                    tmp = rtp.tile([128, t], BF16, tag="rtmp")
                    nc.vector.tensor_mul(tmp[0:32], raw[32:64], sinn_sb[32:64])
                    nc.vector.tensor_mul(tmp[32:64], raw[0:32], sinp_sb[0:32])
                    nc.vector.tensor_mul(tmp[64:96], raw[96:128],
                                         sinn_sb[96:128])
                    nc.vector.tensor_mul(tmp[96:128], raw[64:96],
                                         sinp_sb[64:96])
                    nc.vector.tensor_mul(dst[:], raw[:], cos_sb[:])
                    nc.vector.tensor_add(dst[:], dst[:], tmp[:])

                raws = {}

                def make_qk(pr2):
                    """Work items producing qT[pr2]/kT[pr2]."""
                    rq = praw.tile([128, t], BF16, tag="praw",
                                   name=f"rq{pr2}")
                    rk = praw.tile([128, t], BF16, tag="praw",
                                   name=f"rk{pr2}")
                    items = []
                    for n in range(4):
                        items.append(lambda n=n, rq=rq:
                                     qk_chain(wqT_sb, rq, pr2, n))
                    items.append(lambda rq=rq: rope(rq, qT[pr2]))
                    for n in range(4):
                        items.append(lambda n=n, rk=rk:
                                     qk_chain(wkT_sb, rk, pr2, n))
                    items.append(lambda rk=rk: rope(rk, kT[pr2]))
                    return items

                # ---------- front: qk(pr0) + v(0..3) ----------
                for it in make_qk(0):
                    it()
                for tt in range(4):
                    v_chain(tt)

                # ---------- attention with woven projections ----------
                work = deque()
                pending_epi = None

                def emit_epi(pr, nq, at, dn):
                    nc.vector.reciprocal_approx_fast(out=rcf[0:33, :],
                                                     in_=dn[0:33, :])
                    nc.vector.tensor_copy(rc2[0:33, :], rcf[0:33, :])
                    bc = dnb.tile([128, 512], F32, tag="dn")
                    nc.tensor.matmul(bc[0:64, :], onesbc[0:1, 0:64],
                                     rc2[0:1, :], start=True, stop=True,
                                     tile_position=(0, 0))
                    nc.tensor.matmul(bc[64:128, :], onesbc[32:33, 0:64],
                                     rc2[32:33, :], start=True, stop=True,
                                     tile_position=(32, 64))
                    # DVE can't take two PSUM operands: stage bc in SBUF
                    nc.vector.tensor_copy(bcs[:, :], bc[:, :])
                    nc.vector.tensor_mul(
                        attT[pr][:, 512 * nq:512 * (nq + 1)], at[:, :],
                        bcs[:, :])

                for pr in range(npair):
                    if pr == 0:
                        for tt in range(4, nkc):
                            work.append(lambda tt=tt: v_chain(tt))
                    if pr < npair - 1:
                        work.extend(make_qk(pr + 1))
                    for nq in range(nnq):
                        if pr == npair - 1 and nq >= 1:
                            for tt in range(4 * (nq - 1), 4 * nq):
                                for n2 in (0, 512):
                                    work.append(lambda tt=tt, n2=n2:
                                                wo_chain(tt, n2))
                        K = 4 * nq + 4
                        at = atp.tile([128, 512], F32, tag="atp")
                        dn = dnb.tile([128, 512], F32, tag="dn")
                        exl = {}
                        q0 = 512 * nq
                        h0, h1 = 2 * pr * hd, (2 * pr + 1) * hd
                        for i in range(K + 2):
                            if i < K:
                                kc = i
                                g = 128 * (kc - 4 * nq) if kc >= 4 * nq else 0
                                st = stp.tile([128, 1024], F32, tag="st")
                                nc.tensor.matmul(
                                    st[:, g:512],
                                    kT[pr][0:64, kc * 128:(kc + 1) * 128],
                                    qT[pr][0:64, q0 + g:q0 + 512],
                                    start=True, stop=True,
                                    tile_position=(0, 0))
                                nc.tensor.matmul(
                                    st[:, 512 + g:1024],
                                    kT[pr][64:128, kc * 128:(kc + 1) * 128],
                                    qT[pr][64:128, q0 + g:q0 + 512],
                                    start=True, stop=True,
                                    tile_position=(64, 0))
                                ex = exps.tile([128, 1024], BF16, tag="ex")
                                nc.scalar.activation(ex[:, g:1024],
                                                     st[:, g:1024], EXP,
                                                     scale=float(scale))
                                if kc >= 4 * nq:
                                    # zero the in-block future (q < k) region
                                    nc.gpsimd.affine_select(
                                        out=ex[:, g:512], in_=ex[:, g:512],
                                        compare_op=GE, fill=0.0, base=0,
                                        pattern=[[1, 512 - g]],
                                        channel_multiplier=-1)
                                    nc.gpsimd.affine_select(
                                        out=ex[:, 512 + g:1024],
                                        in_=ex[:, 512 + g:1024],
                                        compare_op=GE, fill=0.0, base=0,
                                        pattern=[[1, 512 - g]],
                                        channel_multiplier=-1)
                                exl[kc] = (ex, g)
                            if i == 1 and pending_epi is not None:
                                emit_epi(*pending_epi)
                                pending_epi = None
                            if i >= 2 and i - 2 < K:
                                kp = i - 2
                                ex, gp = exl.pop(kp)
                                nc.tensor.matmul(
                                    at[0:64, gp:512],
                                    v_sb[kp][:, h0:h0 + hd],
                                    ex[:, gp:512],
                                    start=(kp == 0), stop=(kp == K - 1),
                                    tile_position=(0, 0))
                                nc.tensor.matmul(
                                    at[64:128, gp:512],
                                    v_sb[kp][:, h1:h1 + hd],
                                    ex[:, 512 + gp:1024],
                                    start=(kp == 0), stop=(kp == K - 1),
                                    tile_position=(0, 64))
                                nc.tensor.matmul(
                                    dn[0:1, gp:512], onesd[:, 0:1],
                                    ex[:, gp:512],
                                    start=(kp == 0), stop=(kp == K - 1))
                                nc.tensor.matmul(
                                    dn[32:33, gp:512], onesd[:, 0:1],
                                    ex[:, 512 + gp:1024],
                                    start=(kp == 0), stop=(kp == K - 1))
                            if work and (i % 2 == 1 or i >= K):
                                work.popleft()()
                        pending_epi = (pr, nq, at, dn)

                emit_epi(*pending_epi)
                for tt in range(4 * (nnq - 1), nkc):
                    for n2 in (0, 512):
                        work.append(lambda tt=tt, n2=n2: wo_chain(tt, n2))
                while work:
                    work.popleft()()

    nc.compile()
    return nc


_PROG = None


def _get_program():
    global _PROG
    if _PROG is None:
        _PROG = build_program()
    return _PROG


def _rope_perm(nhc):
    """Per-head row permutation deinterleaving (re, im) pairs."""
    p = []
    for h in range(nhc):
        base = h * HEAD_DIM
        p.extend(base + 2 * i for i in range(HEAD_DIM // 2))
        p.extend(base + 2 * i + 1 for i in range(HEAD_DIM // 2))
    return np.array(p)


def make_core_inputs(x, position_ids, wq, wk, wv, wo, freqs_cos, freqs_sin):
    """Shard + pre-layout the full inputs for the 8 cores."""
    fc = np.asarray(freqs_cos, np.float32)
    fs = np.asarray(freqs_sin, np.float32)
    pos = np.asarray(position_ids)
    perm = _rope_perm(N_HEADS)
    wq_p = np.asarray(wq, np.float32)[perm]
    wk_p = np.asarray(wk, np.float32)[perm]
    wv_ = np.asarray(wv, np.float32)
    wo_ = np.asarray(wo, np.float32)

    in_maps = []
    for c in range(N_CORES):
        b, hh = c // 2, c % 2
        hs = slice(hh * 8 * HEAD_DIM, (hh + 1) * 8 * HEAD_DIM)
        cos_b = fc[pos[b]]                    # [T, 32]
        sin_b = fs[pos[b]]
        cosr = np.tile(cos_b.T, (4, 1))       # [128, T]
        sinr = np.tile(sin_b.T, (4, 1))
        in_maps.append({
            "xT": np.ascontiguousarray(
                np.asarray(x[b], np.float32).T).astype(NPBF16),
            "wqT": np.ascontiguousarray(wq_p[hs].T).astype(NPBF16),
            "wkT": np.ascontiguousarray(wk_p[hs].T).astype(NPBF16),
            "wvT": np.ascontiguousarray(wv_[hs].T).astype(NPBF16),
            "woT": np.ascontiguousarray(wo_[:, hs].T).astype(NPBF16),
            "cosr": cosr.astype(NPBF16),
            "sinp": sinr.astype(NPBF16),
            "sinn": (-sinr).astype(NPBF16),
        })
    return in_maps


def kernel(x, position_ids, mask, wq, wk, wv, wo, freqs_cos, freqs_sin,
           trace=False):
    nc = _get_program()
    in_maps = make_core_inputs(x, position_ids, wq, wk, wv, wo,
                               freqs_cos, freqs_sin)
    res = run_bass_kernel_spmd(nc, in_maps, list(range(N_CORES)),
                               trace=trace, trace_cores=[0] if trace else None)
    outs = [res.results[c]["out"] for c in range(N_CORES)]
    full = np.stack([outs[2 * b] + outs[2 * b + 1] for b in range(B)])
    kernel.last_results = res
    return full.astype(np.float32)


# revision 9
# speedup vs baseline: 1.0237x; 1.0237x over previous
"""Trainium2 Bass kernel for CharOffsetAttention (RoPE attention block).

Sharding (8 cores): data-parallel over batch (B=4 -> pairs of cores) x
tensor-parallel over heads (16 heads -> 8 per core).  Each core computes
qkv projections for its 8 heads, rope, causal attention, and a partial
output projection; the host sums the two head-half partials per batch.

On-device layout is head-dim-major throughout ("S-transposed" scheme):
  qT/kT: [512 head-dims, 2048 tokens]  (rope pairs deinterleaved host-side)
  scores: S.T[k_token, q_token] per head, causal blocks only
  out:    attT is directly the lhsT of the wo matmul

This version exploits PE array tiling concurrency (measured 2x on HW):
  - scores: the two heads of a pair run CONCURRENTLY as row-tiles
    (tile_position (0,0)/(64,0), K=64 each) writing the two halves of one
    [128,1024] PSUM tile, so a single wide exp covers both heads.
  - PV: the two heads run concurrently as col-tiles ((0,0)/(0,64), M=64)
    into one PSUM bank whose partition layout IS the attT layout.
  - softmax denominators: ones-lhsT M=1 matmuls, col-tiled into a shared
    bank (partitions 0/32); reciprocal via reciprocal_approx_fast.
  - qkv / wo projection matmul chains are woven into the attention loop's
    spare slots so the PE never idles and HAM stays un-throttled.
All matmuls are bf16 with f32 PSUM accumulation.
"""
import sys
from collections import deque

if '/opt/trn_rl_repo' not in sys.path:
    sys.path.insert(0, '/opt/trn_rl_repo')

import numpy as np
import ml_dtypes

import concourse.bass as bass
import concourse.bacc as bacc
import concourse.tile as tile
import concourse.mybir as mybir
from concourse.bass_utils import run_bass_kernel_spmd

F32 = mybir.dt.float32
BF16 = mybir.dt.bfloat16
NPBF16 = ml_dtypes.bfloat16

# full-problem constants
B, T, D_MODEL, N_HEADS, HEAD_DIM = 4, 2048, 1024, 16, 64
N_CORES = 8


def build_program(t=T, din=D_MODEL, nhc=N_HEADS // 2, debug=False):
    """Build the per-core SPMD program."""
    hd = HEAD_DIM                    # 64
    dh = nhc * hd                    # 512 head dims on this core
    nkc = t // 128                   # 16 k-token chunks
    nnq = t // 512                   # 4 q-blocks
    npair = nhc // 2                 # 4 head pairs
    ndin = din // 128                # 8 contraction tiles
    scale = 1.0 / np.sqrt(hd)

    nc = bacc.Bacc("TRN2", target_bir_lowering=False, debug=debug,
                   num_devices=N_CORES)

    xT_d = nc.dram_tensor("xT", [din, t], BF16, kind="ExternalInput")
    wqT_d = nc.dram_tensor("wqT", [din, dh], BF16, kind="ExternalInput")
    wkT_d = nc.dram_tensor("wkT", [din, dh], BF16, kind="ExternalInput")
    wvT_d = nc.dram_tensor("wvT", [din, dh], BF16, kind="ExternalInput")
    woT_d = nc.dram_tensor("woT", [dh, din], BF16, kind="ExternalInput")
    cos_d = nc.dram_tensor("cosr", [128, t], BF16, kind="ExternalInput")
    sinp_d = nc.dram_tensor("sinp", [128, t], BF16, kind="ExternalInput")
    sinn_d = nc.dram_tensor("sinn", [128, t], BF16, kind="ExternalInput")
    out_d = nc.dram_tensor("out", [t, din], F32, kind="ExternalOutput")

    GE = mybir.AluOpType.is_ge
    EXP = mybir.ActivationFunctionType.Exp

    with tile.TileContext(nc) as tc:
        with tc.tile_pool(name="persist", bufs=1) as pp:
            cos_sb = pp.tile([128, t], BF16, tag="cos")
            sinp_sb = pp.tile([128, t], BF16, tag="sinp")
            sinn_sb = pp.tile([128, t], BF16, tag="sinn")
            nc.sync.dma_start(cos_sb[:], cos_d[:])
            nc.sync.dma_start(sinp_sb[:], sinp_d[:])
            nc.sync.dma_start(sinn_sb[:], sinn_d[:])

            woT_sb = [pp.tile([128, din], BF16, tag=f"woT{i}", name=f"woT{i}")
                      for i in range(npair)]
            for i in range(npair):
                nc.sync.dma_start(woT_sb[i][:], woT_d[i * 128:(i + 1) * 128, :])

            qT = [pp.tile([128, t], BF16, tag=f"qT{c}", name=f"qT{c}")
                  for c in range(npair)]
            kT = [pp.tile([128, t], BF16, tag=f"kT{c}", name=f"kT{c}")
                  for c in range(npair)]
            attT = [pp.tile([128, t], BF16, tag=f"attT{c}", name=f"attT{c}")
                    for c in range(npair)]
            v_sb = [pp.tile([128, dh], BF16, tag=f"v{i}", name=f"v{i}")
                    for i in range(nkc)]
            # ones for the denominator (K=128,M=1) and broadcast (K=1,M=64)
            onesd = pp.tile([128, 1], BF16, tag="onesd")
            onesbc = pp.tile([128, 64], BF16, tag="onesbc")
            nc.gpsimd.memset(onesd[:], 1.0)
            nc.gpsimd.memset(onesbc[:], 1.0)
            rcf = pp.tile([128, 512], F32, tag="rcf")
            rc2 = pp.tile([128, 512], BF16, tag="rc2")
            bcs = pp.tile([128, 512], F32, tag="bcs")

            with (
                tc.tile_pool(name="pin", bufs=1) as pin,
                tc.tile_pool(name="praw", bufs=4) as praw,
                tc.tile_pool(name="rtp", bufs=2) as rtp,
                tc.tile_pool(name="exps", bufs=6) as exps,
                tc.tile_pool(name="osb", bufs=3) as osb,
                tc.tile_pool(name="stp", bufs=2, space="PSUM") as stp,
                tc.tile_pool(name="atp", bufs=2, space="PSUM") as atp,
                tc.tile_pool(name="dnb", bufs=1, space="PSUM") as dnb,
                tc.tile_pool(name="msc", bufs=1, space="PSUM") as msc,
            ):
                xT_sb = [pin.tile([128, t], BF16, tag=f"xT{k}", name=f"xTs{k}")
                         for k in range(ndin)]
                wqT_sb = [pin.tile([128, dh], BF16, tag=f"wqT{k}",
                                   name=f"wqTs{k}") for k in range(ndin)]
                wkT_sb = [pin.tile([128, dh], BF16, tag=f"wkT{k}",
                                   name=f"wkTs{k}") for k in range(ndin)]
                wvT_sb = [pin.tile([128, dh], BF16, tag=f"wvT{k}",
                                   name=f"wvTs{k}") for k in range(ndin)]
                for k in range(ndin):
                    sl = slice(k * 128, (k + 1) * 128)
                    nc.sync.dma_start(xT_sb[k][:], xT_d[sl, :])
                    nc.sync.dma_start(wqT_sb[k][:], wqT_d[sl, :])
                    nc.sync.dma_start(wkT_sb[k][:], wkT_d[sl, :])
                    nc.sync.dma_start(wvT_sb[k][:], wvT_d[sl, :])

                # ---------- projection chain emitters ----------
                _cpn = [0]

                def chain_ps(pool):
                    p, tg = pool or (msc, "msc")
                    _cpn[0] += 1
                    return p.tile([128, 512], F32, tag=tg,
                                  name=f"cps{_cpn[0]}")

                def qk_chain(w_sb, raw, c, n, pool=None):
                    ps = chain_ps(pool)
                    for kk in range(ndin):
                        nc.tensor.matmul(
                            ps[:], w_sb[kk][:, c * 128:(c + 1) * 128],
                            xT_sb[kk][:, n * 512:(n + 1) * 512],
                            start=(kk == 0), stop=(kk == ndin - 1))
                    nc.vector.tensor_copy(raw[:, n * 512:(n + 1) * 512], ps[:])

                def v_chain(tt, pool=None):
                    ps = chain_ps(pool)
                    for kk in range(ndin):
                        nc.tensor.matmul(
                            ps[:], xT_sb[kk][:, tt * 128:(tt + 1) * 128],
                            wvT_sb[kk][:],
                            start=(kk == 0), stop=(kk == ndin - 1))
                    nc.scalar.copy(v_sb[tt][:], ps[:])

                def wo_chain(tt, n2, pool=None):
                    ps = chain_ps(pool)
                    for kk in range(npair):
                        nc.tensor.matmul(
                            ps[:], attT[kk][:, tt * 128:(tt + 1) * 128],
                            woT_sb[kk][:, n2:n2 + 512],
                            start=(kk == 0), stop=(kk == npair - 1))
                    o = osb.tile([128, 512], F32, tag="osb")
                    nc.vector.tensor_copy(o[:], ps[:])
                    nc.sync.dma_start(out_d[tt * 128:(tt + 1) * 128,
                                            n2:n2 + 512], o[:])

                def rope(raw, dst, c0=0, c1=None):
                    # re' = re*cos - im*sin ; im' = im*cos + re*sin
                    # (sin tables repeat every 32 partitions; slice at the
                    #  source operand's base partition)
                    c1 = t if c1 is None else c1
                    cs = slice(c0, c1)
                    tmp = rtp.tile([128, t], BF16, tag="rtmp")
                    nc.vector.tensor_mul(tmp[0:32, cs], raw[32:64, cs],
                                         sinn_sb[32:64, cs])
                    nc.vector.tensor_mul(tmp[32:64, cs], raw[0:32, cs],
                                         sinp_sb[0:32, cs])
                    nc.vector.tensor_mul(tmp[64:96, cs], raw[96:128, cs],
                                         sinn_sb[96:128, cs])
                    nc.vector.tensor_mul(tmp[96:128, cs], raw[64:96, cs],
                                         sinp_sb[64:96, cs])
                    nc.vector.tensor_mul(dst[:, cs], raw[:, cs],
                                         cos_sb[:, cs])
                    nc.vector.tensor_add(dst[:, cs], dst[:, cs], tmp[:, cs])

                raws = {}

                def make_qk(pr2):
                    """Work items producing qT[pr2]/kT[pr2]."""
                    rq = praw.tile([128, t], BF16, tag="praw",
                                   name=f"rq{pr2}")
                    rk = praw.tile([128, t], BF16, tag="praw",
                                   name=f"rk{pr2}")
                    items = []
                    for n in range(4):
                        items.append(lambda pool=None, n=n, rq=rq:
                                     qk_chain(wqT_sb, rq, pr2, n, pool))
                    items.append(lambda pool=None, rq=rq: rope(rq, qT[pr2]))
                    for n in range(4):
                        items.append(lambda pool=None, n=n, rk=rk:
                                     qk_chain(wkT_sb, rk, pr2, n, pool))
                    items.append(lambda pool=None, rk=rk: rope(rk, kT[pr2]))
                    return items

                # ---------- front: qk(pr0) + v(0..3), rope by halves ----------
                rq0 = praw.tile([128, t], BF16, tag="praw", name="rq0")
                rk0 = praw.tile([128, t], BF16, tag="praw", name="rk0")
                fpools = [(msc, "msc"), (atp, "atp"), (dnb, "dn")]
                qk_chain(wqT_sb, rq0, 0, 0, fpools[0])
                qk_chain(wqT_sb, rq0, 0, 1, fpools[1])
                qk_chain(wkT_sb, rk0, 0, 0, fpools[2])
                qk_chain(wkT_sb, rk0, 0, 1, fpools[0])
                rope(rq0, qT[0], 0, 1024)
                rope(rk0, kT[0], 0, 1024)
                qk_chain(wqT_sb, rq0, 0, 2, fpools[1])
                qk_chain(wqT_sb, rq0, 0, 3, fpools[2])
                qk_chain(wkT_sb, rk0, 0, 2, fpools[0])
                qk_chain(wkT_sb, rk0, 0, 3, fpools[1])
                rope(rq0, qT[0], 1024, t)
                rope(rk0, kT[0], 1024, t)
                for tt in range(4):
                    v_chain(tt, fpools[(2 + tt) % 3])

                # ---------- attention with woven projections ----------
                work = deque()
                pending_epi = None

                def emit_epi(pr, nq, at, dn):
                    nc.vector.reciprocal_approx_fast(out=rcf[0:33, :],
                                                     in_=dn[0:33, :])
                    nc.vector.tensor_copy(rc2[0:33, :], rcf[0:33, :])
                    bc = dnb.tile([128, 512], F32, tag="dn")
                    nc.tensor.matmul(bc[0:64, :], onesbc[0:1, 0:64],
                                     rc2[0:1, :], start=True, stop=True,
                                     tile_position=(0, 0))
                    nc.tensor.matmul(bc[64:128, :], onesbc[32:33, 0:64],
                                     rc2[32:33, :], start=True, stop=True,
                                     tile_position=(32, 64))
                    # DVE can't take two PSUM operands: stage bc in SBUF
                    nc.vector.tensor_copy(bcs[:, :], bc[:, :])
                    nc.vector.tensor_mul(
                        attT[pr][:, 512 * nq:512 * (nq + 1)], at[:, :],
                        bcs[:, :])

                TR = 3                       # PV trails scores by TR kcs
                slots_per_nq = [
                    sum(1 for i in range(4 * nq + 4 + TR)
                        if i % 2 == 1 or i >= 4 * nq + 4)
                    for nq in range(nnq)]
                for pr in range(npair):
                    if pr == 0:
                        for tt in range(4, nkc):
                            work.append(lambda pool=None, tt=tt:
                                        v_chain(tt, pool))
                    if pr < npair - 1:
                        work.extend(make_qk(pr + 1))
                    slots_left = sum(slots_per_nq)
                    emitted = skipped = 0
                    for nq in range(nnq):
                        if pr == npair - 1 and nq >= 1:
                            for tt in range(4 * (nq - 1), 4 * nq):
                                for n2 in (0, 512):
                                    work.append(lambda pool=None, tt=tt,
                                                n2=n2: wo_chain(tt, n2, pool))
                        K = 4 * nq + 4
                        at = atp.tile([128, 512], F32, tag="atp")
                        dn = dnb.tile([128, 512], F32, tag="dn")
                        exl = {}
                        q0 = 512 * nq
                        h0, h1 = 2 * pr * hd, (2 * pr + 1) * hd
                        for i in range(K + TR):
                            if i < K:
                                kc = i
                                g = 128 * (kc - 4 * nq) if kc >= 4 * nq else 0
                                st = stp.tile([128, 1024], F32, tag="st")
                                nc.tensor.matmul(
                                    st[:, g:512],
                                    kT[pr][0:64, kc * 128:(kc + 1) * 128],
                                    qT[pr][0:64, q0 + g:q0 + 512],
                                    start=True, stop=True,
                                    tile_position=(0, 0))
                                nc.tensor.matmul(
                                    st[:, 512 + g:1024],
                                    kT[pr][64:128, kc * 128:(kc + 1) * 128],
                                    qT[pr][64:128, q0 + g:q0 + 512],
                                    start=True, stop=True,
                                    tile_position=(64, 0))
                                ex = exps.tile([128, 1024], BF16, tag="ex")
                                nc.scalar.activation(ex[:, g:1024],
                                                     st[:, g:1024], EXP,
                                                     scale=float(scale))
                                if kc >= 4 * nq:
                                    # zero the in-block future (q < k) region
                                    nc.gpsimd.affine_select(
                                        out=ex[:, g:512], in_=ex[:, g:512],
                                        compare_op=GE, fill=0.0, base=0,
                                        pattern=[[1, 512 - g]],
                                        channel_multiplier=-1)
                                    nc.gpsimd.affine_select(
                                        out=ex[:, 512 + g:1024],
                                        in_=ex[:, 512 + g:1024],
                                        compare_op=GE, fill=0.0, base=0,
                                        pattern=[[1, 512 - g]],
                                        channel_multiplier=-1)
                                exl[kc] = (ex, g)
                            if i == 1 and pending_epi is not None:
                                emit_epi(*pending_epi)
                                pending_epi = None
                            if i >= TR and i - TR < K:
                                kp = i - TR
                                ex, gp = exl.pop(kp)
                                nc.tensor.matmul(
                                    at[0:64, gp:512],
                                    v_sb[kp][:, h0:h0 + hd],
                                    ex[:, gp:512],
                                    start=(kp == 0), stop=(kp == K - 1),
                                    tile_position=(0, 0))
                                nc.tensor.matmul(
                                    at[64:128, gp:512],
                                    v_sb[kp][:, h1:h1 + hd],
                                    ex[:, 512 + gp:1024],
                                    start=(kp == 0), stop=(kp == K - 1),
                                    tile_position=(0, 64))
                                nc.tensor.matmul(
                                    dn[0:1, gp:512], onesd[:, 0:1],
                                    ex[:, gp:512],
                                    start=(kp == 0), stop=(kp == K - 1))
                                nc.tensor.matmul(
                                    dn[32:33, gp:512], onesd[:, 0:1],
                                    ex[:, 512 + gp:1024],
                                    start=(kp == 0), stop=(kp == K - 1))
                            if i % 2 == 1 or i >= K:
                                # pace chain emission evenly across the stage:
                                # emit when pending items >= remaining slots
                                # (classic Bresenham-style spreading)
                                if work and len(work) * (skipped + 1) >= \
                                        slots_left:
                                    work.popleft()()
                                    emitted += 1
                                    skipped = 0
                                else:
                                    skipped += 1
                                slots_left -= 1
                        pending_epi = (pr, nq, at, dn)

                emit_epi(*pending_epi)
                for tt in range(4 * (nnq - 1), nkc):
                    for n2 in (0, 512):
                        work.append(lambda pool=None, tt=tt, n2=n2:
                                    wo_chain(tt, n2, pool))
                ti = 0
                while work:
                    work.popleft()(fpools[ti % 3])
                    ti += 1

    nc.compile()
    return nc


_PROG = None


def _get_program():
    global _PROG
    if _PROG is None:
        _PROG = build_program()
    return _PROG


def _rope_perm(nhc):
    """Per-head row permutation deinterleaving (re, im) pairs."""
    p = []
    for h in range(nhc):
        base = h * HEAD_DIM
        p.extend(base + 2 * i for i in range(HEAD_DIM // 2))
        p.extend(base + 2 * i + 1 for i in range(HEAD_DIM // 2))
    return np.array(p)


def make_core_inputs(x, position_ids, wq, wk, wv, wo, freqs_cos, freqs_sin):
    """Shard + pre-layout the full inputs for the 8 cores."""
    fc = np.asarray(freqs_cos, np.float32)
    fs = np.asarray(freqs_sin, np.float32)
    pos = np.asarray(position_ids)
    perm = _rope_perm(N_HEADS)
    wq_p = np.asarray(wq, np.float32)[perm]
    wk_p = np.asarray(wk, np.float32)[perm]
    wv_ = np.asarray(wv, np.float32)
    wo_ = np.asarray(wo, np.float32)

    in_maps = []
    for c in range(N_CORES):
        b, hh = c // 2, c % 2
        hs = slice(hh * 8 * HEAD_DIM, (hh + 1) * 8 * HEAD_DIM)
        cos_b = fc[pos[b]]                    # [T, 32]
        sin_b = fs[pos[b]]
        cosr = np.tile(cos_b.T, (4, 1))       # [128, T]
        sinr = np.tile(sin_b.T, (4, 1))
        in_maps.append({
            "xT": np.ascontiguousarray(
                np.asarray(x[b], np.float32).T).astype(NPBF16),
            "wqT": np.ascontiguousarray(wq_p[hs].T).astype(NPBF16),
            "wkT": np.ascontiguousarray(wk_p[hs].T).astype(NPBF16),
            "wvT": np.ascontiguousarray(wv_[hs].T).astype(NPBF16),
            "woT": np.ascontiguousarray(wo_[:, hs].T).astype(NPBF16),
            "cosr": cosr.astype(NPBF16),
            "sinp": sinr.astype(NPBF16),
            "sinn": (-sinr).astype(NPBF16),
        })
    return in_maps


def kernel(x, position_ids, mask, wq, wk, wv, wo, freqs_cos, freqs_sin,
           trace=False):
    nc = _get_program()
    in_maps = make_core_inputs(x, position_ids, wq, wk, wv, wo,
                               freqs_cos, freqs_sin)
    res = run_bass_kernel_spmd(nc, in_maps, list(range(N_CORES)),
                               trace=trace, trace_cores=[0] if trace else None)
    outs = [res.results[c]["out"] for c in range(N_CORES)]
    full = np.stack([outs[2 * b] + outs[2 * b + 1] for b in range(B)])
    kernel.last_results = res
    return full.astype(np.float32)
